# revision 10
# baseline (speedup 1.0000x reference)
"""Trainium2 Bass kernel for nn_Entity_Classify (RGCN-style entity classification).

Algorithm (8 NeuronCores, SPMD):
  - Destination nodes are permuted into 392 windows of 128 slots, balanced so
    every (relation, window, src-half) has <= 128 incoming edges. Each core owns
    49 consecutive windows for ALL 8 relations -> no output collective needed.
  - Launch 1: each core builds its shard of x = sum_f emb[f][codes[f]] via a
    dma_gather from the flattened embedding table; host assembles the full x
    table in natural node order.
  - Launch 2 (per core, per window group): dma_gather x[src] for the group's
    edges (two gathers: src<25088 from the table head, src>=25088 from an
    offset view, keeping int16 indices in range); build one-hot H from dst
    slots on-device (iota + is_equal); segment-sum via PE matmuls
    msg = H^T @ xg and deg = H^T @ 1 (H stays weight-stationary);
    mean = msg * 1/max(deg,1) as a per-partition scale; PE transpose;
    h_pre^T accumulates sum_r Wn_r^T @ mean_r^T + Ws_sum^T @ x^T in PSUM;
    relu(+bias) on ACT; classifier matmul; host un-permutes the outputs.
  All floating-point math (embedding sum, W_neigh = coeff@basis, Ws_sum,
  biases, segment mean, matmuls, relu, classifier) happens on device.
"""

import os
import sys

import numpy as np

try:
    import concourse.bass as bass  # noqa: F401
except ImportError:  # pragma: no cover
    sys.path.insert(0, "/opt/trn_rl_repo")
    import concourse.bass as bass  # noqa: F401

import concourse.mybir as mybir
import concourse.tile as tile
from concourse import bacc
from concourse.bass_utils import run_bass_kernel_spmd
from concourse.masks import make_identity

F32 = mybir.dt.float32
I32 = mybir.dt.int32
I16 = mybir.dt.int16

# Problem sizes (hardcoded from the spec).
N = 50000
D = 128
R = 8
E = 80000
B = 4
NF = 4
V = 1000
OUT = 16

NCORES = 8
P = 128
WIN_TOTAL = 392
WIN_PER_CORE = 49
N_PAD = WIN_TOTAL * P  # 50176 positions
POS_PER_CORE = WIN_PER_CORE * P  # 6272
GROUPW = 3  # windows per group in the main loop
NGROUPS = 17  # 16 groups of 3 windows + 1 group of 1 window
LASTW = WIN_PER_CORE - GROUPW * (NGROUPS - 1)  # 1

# x table (natural node order) and the int16-safe lo/hi split.
NTAB = 50048  # 50000 nodes + zero pad rows
SPLIT = 25088  # src < SPLIT -> lo gather, else hi gather
HIOFF = NTAB - 32768  # 17280; hi gather reads x[HIOFF:HIOFF+32768]

_ALU = mybir.AluOpType
_ACT = mybir.ActivationFunctionType


def _group_w(g):
    return GROUPW if g < NGROUPS - 1 else LASTW


def _col_layout():
    """Per-group column bases. Per group: 2*R*wg chunk columns
    (lo chunks first: r*wg+wi, then hi chunks at R*wg + r*wg+wi)."""
    bases = []
    acc = 0
    for g in range(NGROUPS):
        bases.append(acc)
        acc += 2 * R * _group_w(g)
    return bases, acc


COL_BASES, NCOLS = _col_layout()  # NCOLS == 784
# int16 index-column offsets per group per half (16 idxs per column).
SOFF = [R * _group_w(g) * 8 for g in range(NGROUPS)]
SOFF = [0] + list(np.cumsum(SOFF))
S_TOT = SOFF[-1]  # 3136


def _pack16(dst, j_arr, s_off):
    """Pack indices j_arr (order j) into int16 grid at [16k + j%16, s_off + j//16],
    replicated across the 8 Q7 partition groups."""
    jpos = np.arange(len(j_arr))
    for k in range(8):
        dst[16 * k + jpos % 16, s_off + jpos // 16] = j_arr


# ---------------------------------------------------------------------------
# Host-side preparation (integer index bookkeeping only)
# ---------------------------------------------------------------------------


def _assign_positions(deg, cap):
    """Assign nodes to padded positions so each window's per-column count of
    `deg` [N, K] stays <= cap. Returns node_at_pos [N_PAD] (-1 = empty)."""
    K = deg.shape[1]
    deg_pad = np.vstack([deg, np.zeros((N_PAD - N, K), np.int64)])
    rng = np.random.default_rng(12345)
    for _attempt in range(8):
        perm = rng.permutation(N)
        node_at_pos = np.full(N_PAD, -1, np.int64)
        node_at_pos[:N] = perm
        d2 = np.zeros((N_PAD, K), np.int64)
        d2[:N] = deg[perm]
        cnt = d2.reshape(WIN_TOTAL, P, K).sum(1)

        ok = True
        for _it in range(6000):
            over = cnt.max(1) - cap
            w1 = int(np.argmax(over))
            if over[w1] <= 0:
                break
            r1 = int(np.argmax(cnt[w1]))
            nd = node_at_pos[w1 * P : (w1 + 1) * P]
            degs1 = np.where(nd >= 0, deg_pad[np.clip(nd, 0, None), r1], -1)
            p1 = int(np.argmax(degs1))
            n1 = nd[p1]
            dn1 = deg_pad[n1]
            newmax = (cnt + dn1).max(1)
            newmax[w1] = 1 << 30
            w2 = int(np.argmin(newmax))
            nd2 = node_at_pos[w2 * P : (w2 + 1) * P]
            degs2 = np.where(nd2 >= 0, deg_pad[np.clip(nd2, 0, None), r1], 0)
            p2 = int(np.argmin(degs2))
            n2 = nd2[p2]
            dn2 = deg_pad[n2] if n2 >= 0 else np.zeros(K, np.int64)
            node_at_pos[w1 * P + p1] = n2
            node_at_pos[w2 * P + p2] = n1
            cnt[w1] += dn2 - dn1
            cnt[w2] += dn1 - dn2
        else:
            ok = False
        if ok and (cnt <= cap).all():
            return node_at_pos
    raise RuntimeError("window balancing failed")


def _prepare(inputs):
    """All host-side integer bookkeeping."""
    feat_codes = np.asarray(inputs["feat_codes"])
    nid = np.asarray(inputs["nid"])
    edges_src = np.asarray(inputs["edges_src"])
    edges_dst = np.asarray(inputs["edges_dst"])

    # per-node degree split by (relation, src-half)
    deg = np.zeros((N, 2 * R), np.int64)
    for r in range(R):
        half = (np.asarray(edges_src[r]) >= SPLIT).astype(np.int64)
        np.add.at(deg[:, 2 * r], edges_dst[r][half == 0], 1)
        np.add.at(deg[:, 2 * r + 1], edges_dst[r][half == 1], 1)
    node_at_pos = _assign_positions(deg, P)

    pos_of_node = np.full(N, -1, np.int64)
    valid = node_at_pos >= 0
    pos_of_node[node_at_pos[valid]] = np.nonzero(valid)[0]

    # Packed edge arrays per core.
    srclo = np.zeros((NCORES, P, S_TOT), np.int16)
    srchi = np.zeros((NCORES, P, S_TOT), np.int16)
    dstloc = np.full((NCORES, P, NCOLS), -1.0, np.float32)
    for r in range(R):
        src = np.asarray(edges_src[r]).astype(np.int64)
        dpos = pos_of_node[edges_dst[r]]
        w = dpos // P
        slot = (dpos % P).astype(np.float32)
        half = (src >= SPLIT).astype(np.int64)
        order = np.lexsort((half, w))
        ws = w[order]
        bounds = np.searchsorted(ws, np.arange(WIN_TOTAL + 1))
        for wglob in range(WIN_TOTAL):
            lo_, hi_ = bounds[wglob], bounds[wglob + 1]
            if lo_ == hi_:
                continue
            core, wl = divmod(wglob, WIN_PER_CORE)
            g = min(wl // GROUPW, NGROUPS - 1)
            wi = wl - g * GROUPW
            wg = _group_w(g)
            ed = order[lo_:hi_]
            hmask = half[ed]
            for h, (arr, bias) in enumerate(((srclo, 0), (srchi, HIOFF))):
                seg = ed[hmask == h]
                assert len(seg) <= P, f"window overflow {wglob} r={r} h={h}"
                if len(seg) == 0:
                    continue
                cwh = r * wg + wi  # chunk-column within this half
                jpos = cwh * P + np.arange(len(seg))
                vals = (src[seg] - bias).astype(np.int16)
                for k in range(8):
                    arr[core, 16 * k + jpos % 16, SOFF[g] + jpos // 16] = vals
                col = COL_BASES[g] + h * R * wg + cwh
                dstloc[core, : len(seg), col] = slot[seg]

    # Launch-1 gather indices (natural node order, flat emb row f*V + code).
    codes = feat_codes[:, nid].astype(np.int64)  # [NF, N]
    idx16 = np.zeros((NCORES, P, NF * WIN_PER_CORE * 8), np.int16)
    for core in range(NCORES):
        base_node = core * POS_PER_CORE
        nodes = base_node + np.arange(POS_PER_CORE)
        ok = nodes < N
        safe = np.clip(nodes, 0, N - 1)
        for f in range(NF):
            vals = np.where(ok, f * V + codes[f][safe], 0)
            # j = (f*49 + c)*128 + p  with node = base + c*128 + p
            _pack16(idx16[core], vals.astype(np.int16), f * WIN_PER_CORE * 8)
    return dict(
        node_at_pos=node_at_pos,
        srclo=srclo,
        srchi=srchi,
        dstloc=dstloc,
        idx16=idx16,
    )


# ---------------------------------------------------------------------------
# Launch 1: embedding build (x shard per core, natural node order)
# ---------------------------------------------------------------------------


def build_l1():
    nc = bacc.Bacc(None, target_bir_lowering=False,
                   dynamic_dma_scratch_size=16384, num_swdge_queues=2)
    emb_d = nc.dram_tensor("emb", [NF * V, D], F32, kind="ExternalInput")
    idx_d = nc.dram_tensor("idx16", [P, NF * WIN_PER_CORE * 8], I16,
                           kind="ExternalInput")
    out_d = nc.dram_tensor("xout", [P, WIN_PER_CORE, D], F32,
                           kind="ExternalOutput")

    W = WIN_PER_CORE
    with tile.TileContext(nc) as tc:
        with tc.tile_pool(name="sb", bufs=1) as sb:
            idx_t = sb.tile([P, NF * W * 8], I16)
            nc.sync.dma_start(idx_t[:, :], idx_d[:, :])
            g_t = sb.tile([P, NF * W, D], F32)
            for f in range(NF):
                nc.gpsimd.dma_gather(
                    out_ap=g_t[:, f * W : (f + 1) * W, :], in_ap=emb_d[:, :],
                    idxs_ap=idx_t[:, f * W * 8 : (f + 1) * W * 8],
                    num_idxs=W * P, num_idxs_reg=W * P, elem_size=D,
                    single_packet=False, queue_num=f % 2,
                )
            t01 = sb.tile([P, W, D], F32)
            x_t = sb.tile([P, W, D], F32)
            nc.vector.tensor_tensor(
                out=t01[:, :, :], in0=g_t[:, 0:W, :], in1=g_t[:, W : 2 * W, :],
                op=_ALU.add,
            )
            nc.vector.tensor_tensor(
                out=x_t[:, :, :], in0=t01[:, :, :], in1=g_t[:, 2 * W : 3 * W, :],
                op=_ALU.add,
            )
            nc.vector.tensor_tensor(
                out=x_t[:, :, :], in0=x_t[:, :, :], in1=g_t[:, 3 * W : 4 * W, :],
                op=_ALU.add,
            )
            # reuse the gather buffer region for output staging is not needed;
            # write x_t through the first W blocks of out_d
            nc.sync.dma_start(out_d[:, :, :], x_t[:, :, :])
    return nc


# ---------------------------------------------------------------------------
# Launch 2: main compute
# ---------------------------------------------------------------------------


def build_l2():
    nc = bacc.Bacc(None, target_bir_lowering=False,
                   dynamic_dma_scratch_size=16384, num_swdge_queues=2)
    x_d = nc.dram_tensor("x", [NTAB, D], F32, kind="ExternalInput")
    xwin_d = nc.dram_tensor("xwin", [P, WIN_PER_CORE, D], F32, kind="ExternalInput")
    srclo_d = nc.dram_tensor("srclo", [P, S_TOT], I16, kind="ExternalInput")
    srchi_d = nc.dram_tensor("srchi", [P, S_TOT], I16, kind="ExternalInput")
    dst_d = nc.dram_tensor("dst", [P, NCOLS], F32, kind="ExternalInput")
    basis_d = nc.dram_tensor("basis", [B, D, D], F32, kind="ExternalInput")
    coeff_d = nc.dram_tensor("coeff", [1, R * B], F32, kind="ExternalInput")
    wself_d = nc.dram_tensor("wself", [R, D, D], F32, kind="ExternalInput")
    brel_d = nc.dram_tensor("brel", [R, D], F32, kind="ExternalInput")
    hbias_d = nc.dram_tensor("hbias", [1, D], F32, kind="ExternalInput")
    wcls_d = nc.dram_tensor("wcls", [D, OUT], F32, kind="ExternalInput")
    bcls_d = nc.dram_tensor("bcls", [1, OUT], F32, kind="ExternalInput")
    hT_d = nc.dram_tensor("hT", [P, WIN_PER_CORE, P], F32, kind="ExternalOutput")
    lgT_d = nc.dram_tensor("lgT", [OUT, WIN_PER_CORE, P], F32, kind="ExternalOutput")

    with tile.TileContext(nc) as tc:
        with (
            tc.tile_pool(name="const", bufs=1) as cpool,
            tc.tile_pool(name="xg", bufs=2) as xgpool,
            tc.tile_pool(name="hh", bufs=2) as hpool,
            tc.tile_pool(name="small", bufs=3) as spool,
            tc.tile_pool(name="mean", bufs=2) as mpool,
            tc.tile_pool(name="mts", bufs=2) as mtspool,
            tc.tile_pool(name="pmsg", bufs=2, space="PSUM") as pmsg,
            tc.tile_pool(name="pmt", bufs=2, space="PSUM") as pmt,
            tc.tile_pool(name="ph", bufs=2, space="PSUM") as ph,
            tc.tile_pool(name="plg", bufs=2, space="PSUM") as plg,
        ):
            # ---- constants / weights prep ----
            ident = cpool.tile([P, P], F32)
            make_identity(nc, ident[:, :])
            iota_i = cpool.tile([P, P], I32)
            nc.gpsimd.iota(iota_i[:, :], pattern=[[1, P]], base=0, channel_multiplier=0)
            iota_f = cpool.tile([P, P], F32)
            nc.vector.tensor_copy(iota_f[:, :], iota_i[:, :])

            srclo_t = cpool.tile([P, S_TOT], I16)
            srchi_t = cpool.tile([P, S_TOT], I16)
            dst_t = cpool.tile([P, NCOLS], F32)
            nc.sync.dma_start(srclo_t[:, :], srclo_d[:, :])
            nc.sync.dma_start(srchi_t[:, :], srchi_d[:, :])
            nc.sync.dma_start(dst_t[:, :], dst_d[:, :])

            basis_t = cpool.tile([P, B, D], F32)
            nc.sync.dma_start(basis_t[:, :, :], basis_d.rearrange("b i o -> i b o"))
            wself_t = cpool.tile([P, R, D], F32)
            nc.sync.dma_start(wself_t[:, :, :], wself_d.rearrange("r i o -> i r o"))
            coeff_t = cpool.tile([1, R * B], F32)
            nc.sync.dma_start(coeff_t[:, :], coeff_d[:, :])
            brel_t = cpool.tile([R, D], F32)
            nc.sync.dma_start(brel_t[:, :], brel_d[:, :])
            hbias_t = cpool.tile([1, D], F32)
            nc.sync.dma_start(hbias_t[:, :], hbias_d[:, :])
            wcls_t = cpool.tile([D, OUT], F32)
            nc.sync.dma_start(wcls_t[:, :], wcls_d[:, :])
            bcls_t = cpool.tile([1, OUT], F32)
            nc.sync.dma_start(bcls_t[:, :], bcls_d[:, :])

            ones_row = cpool.tile([1, P], F32)
            nc.vector.memset(ones_row[:, :], 1.0)
            ones_col = cpool.tile([P, 1], F32)
            nc.vector.memset(ones_col[:, :], 1.0)
            ones8 = cpool.tile([R, 1], F32)
            nc.vector.memset(ones8[:, :], 1.0)
            one1 = cpool.tile([1, 1], F32)
            nc.vector.memset(one1[:, :], 1.0)

            # broadcast coeff to all partitions via ones matmul
            cb_ps = pmt.tile([P, R * B], F32, tag="mt")
            nc.tensor.matmul(cb_ps[:, :], ones_row[:, :], coeff_t[:, :],
                             start=True, stop=True)
            coeffb = cpool.tile([P, R * B], F32)
            nc.vector.tensor_copy(coeffb[:, :], cb_ps[:, :])

            # W_neigh[r] = sum_b coeff[r,b] * basis[b]  -> [i, r, o]
            wn_t = cpool.tile([P, R, D], F32)
            tmp_w = cpool.tile([P, D], F32)
            for r in range(R):
                eng = nc.vector if r % 2 == 0 else nc.gpsimd
                eng.tensor_scalar(
                    out=wn_t[:, r, :], in0=basis_t[:, 0, :],
                    scalar1=coeffb[:, 4 * r : 4 * r + 1], scalar2=None, op0=_ALU.mult,
                )
                for b in range(1, B):
                    eng.tensor_scalar(
                        out=tmp_w[:, :], in0=basis_t[:, b, :],
                        scalar1=coeffb[:, 4 * r + b : 4 * r + b + 1], scalar2=None,
                        op0=_ALU.mult,
                    )
                    eng.tensor_tensor(
                        out=wn_t[:, r, :], in0=wn_t[:, r, :], in1=tmp_w[:, :],
                        op=_ALU.add,
                    )

            # Ws_sum = sum_r W_self[r]
            wssum = cpool.tile([P, D], F32)
            nc.vector.tensor_tensor(
                out=wssum[:, :], in0=wself_t[:, 0, :], in1=wself_t[:, 1, :], op=_ALU.add
            )
            for r in range(2, R):
                nc.vector.tensor_tensor(
                    out=wssum[:, :], in0=wssum[:, :], in1=wself_t[:, r, :], op=_ALU.add
                )

            # b_total^T [P,1] = (sum_r b_rel[r] + h_bias)^T ; b_cls^T [OUT,1]
            bsum_ps = plg.tile([1, D], F32, tag="lg")
            nc.tensor.matmul(bsum_ps[:, :], ones8[:, :], brel_t[:, :],
                             start=True, stop=True)
            bb = cpool.tile([1, D], F32)
            nc.vector.tensor_tensor(
                out=bb[:, :], in0=bsum_ps[:, :], in1=hbias_t[:, :], op=_ALU.add
            )
            btot_ps = pmt.tile([P, 1], F32, tag="mt")
            nc.tensor.matmul(btot_ps[:, :], bb[:, :], one1[:, :], start=True, stop=True)
            btotT = cpool.tile([P, 1], F32)
            nc.vector.tensor_copy(btotT[:, :], btot_ps[:, :])
            bcls_ps = plg.tile([OUT, 1], F32, tag="lg")
            nc.tensor.matmul(bcls_ps[:, :], bcls_t[:, :], one1[:, :],
                             start=True, stop=True)
            bclsT = cpool.tile([OUT, 1], F32)
            nc.vector.tensor_copy(bclsT[:, :], bcls_ps[:, :])

            # xT for the self path: transpose each of the core's x windows.
            xwin_t = hpool.tile([P, WIN_PER_CORE, D], F32, tag="H")
            nc.sync.dma_start(xwin_t[:, :, :], xwin_d[:, :, :])
            xT = cpool.tile([P, WIN_PER_CORE, D], F32)
            for w in range(WIN_PER_CORE):
                tp = pmt.tile([P, GROUPW, P], F32, tag="mt")
                nc.tensor.transpose(tp[:, 0, :], xwin_t[:, w, :], ident[:, :])
                if w % 2 == 0:
                    nc.vector.tensor_copy(xT[:, w, :], tp[:, 0, :])
                else:
                    nc.scalar.activation(xT[:, w, :], tp[:, 0, :], _ACT.Copy)

            hT_sb = cpool.tile([P, WIN_PER_CORE, P], F32)
            lgT_sb = cpool.tile([OUT, WIN_PER_CORE, P], F32)

            # ---- main loop over window groups ----
            for g in range(NGROUPS):
                wg = _group_w(g)
                nchunk = R * wg  # chunk columns per half
                base = COL_BASES[g]

                xg = xgpool.tile([P, 2 * R * GROUPW, P], F32, tag="xg")
                nc.gpsimd.dma_gather(
                    out_ap=xg[:, 0:nchunk, :], in_ap=x_d[0:32768, :],
                    idxs_ap=srclo_t[:, SOFF[g] : SOFF[g] + nchunk * 8],
                    num_idxs=nchunk * P, num_idxs_reg=nchunk * P, elem_size=D,
                    single_packet=False, queue_num=0,
                )
                nc.gpsimd.dma_gather(
                    out_ap=xg[:, nchunk : 2 * nchunk, :],
                    in_ap=x_d[HIOFF : HIOFF + 32768, :],
                    idxs_ap=srchi_t[:, SOFF[g] : SOFF[g] + nchunk * 8],
                    num_idxs=nchunk * P, num_idxs_reg=nchunk * P, elem_size=D,
                    single_packet=False, queue_num=1,
                )

                H = hpool.tile([P, 2 * R * GROUPW, P], F32, tag="H")
                for lc in range(2 * nchunk):
                    col = base + lc
                    eng = nc.vector if lc % 2 == 0 else nc.gpsimd
                    eng.tensor_scalar(
                        out=H[:, lc, :], in0=iota_f[:, :],
                        scalar1=dst_t[:, col : col + 1], scalar2=None,
                        op0=_ALU.is_equal,
                    )

                h_ps = ph.tile([P, GROUPW, P], F32, tag="h")
                nc.tensor.matmul(
                    h_ps[:, 0:wg, :], wssum[:, :],
                    xT[:, g * GROUPW : g * GROUPW + wg, :], start=True, stop=False,
                )

                for r in range(R):
                    msg = pmsg.tile([P, GROUPW, 129], F32, tag="msg")
                    for wi in range(wg):
                        lc_lo = r * wg + wi
                        lc_hi = nchunk + r * wg + wi
                        # One accumulation group per msg tile: a single
                        # start=True clears the bank's has_written bits; the
                        # per-element bits then make every first write to a
                        # region an overwrite and later ones accumulates.
                        nc.tensor.matmul(
                            msg[:, wi, 0:128], H[:, lc_lo, :], xg[:, lc_lo, :],
                            start=(wi == 0), stop=False, skip_group_check=True,
                        )
                        nc.tensor.matmul(
                            msg[:, wi, 128:129], H[:, lc_lo, :], ones_col[:, :],
                            start=False, stop=False, skip_group_check=True,
                        )
                        nc.tensor.matmul(
                            msg[:, wi, 0:128], H[:, lc_hi, :], xg[:, lc_hi, :],
                            start=False, stop=False, skip_group_check=True,
                        )
                        nc.tensor.matmul(
                            msg[:, wi, 128:129], H[:, lc_hi, :], ones_col[:, :],
                            start=False, stop=(wi == wg - 1), skip_group_check=True,
                        )
                    degm = spool.tile([P, GROUPW], F32, tag="degm")
                    nc.vector.tensor_scalar(
                        out=degm[:, 0:wg], in0=msg[:, 0:wg, 128], scalar1=1.0,
                        scalar2=None, op0=_ALU.max,
                    )
                    recip = spool.tile([P, GROUPW], F32, tag="recip")
                    nc.vector.reciprocal(recip[:, 0:wg], degm[:, 0:wg])
                    mean = mpool.tile([P, GROUPW, P], F32, tag="mean")
                    nc.vector.tensor_tensor(
                        out=mean[:, 0:wg, :], in0=msg[:, 0:wg, 0:128],
                        in1=recip[:, 0:wg].to_broadcast([P, wg, P]), op=_ALU.mult,
                    )
                    mt_ps = pmt.tile([P, GROUPW, P], F32, tag="mt")
                    for wi in range(wg):
                        nc.tensor.transpose(mt_ps[:, wi, :], mean[:, wi, :],
                                            ident[:, :])
                    meanT = mtspool.tile([P, GROUPW, P], F32, tag="mts")
                    if r % 2 == 0:
                        nc.vector.tensor_copy(meanT[:, 0:wg, :], mt_ps[:, 0:wg, :])
                    else:
                        nc.scalar.activation(meanT[:, 0:wg, :], mt_ps[:, 0:wg, :],
                                             _ACT.Copy)
                    nc.tensor.matmul(
                        h_ps[:, 0:wg, :], wn_t[:, r, :], meanT[:, 0:wg, :],
                        start=False, stop=(r == R - 1),
                    )

                nc.scalar.activation(
                    hT_sb[:, g * GROUPW : g * GROUPW + wg, :], h_ps[:, 0:wg, :],
                    _ACT.Relu, bias=btotT[:, 0:1], scale=1.0,
                )
                lg_ps = plg.tile([OUT, GROUPW, P], F32, tag="lg")
                nc.tensor.matmul(
                    lg_ps[:, 0:wg, :], wcls_t[:, :],
                    hT_sb[:, g * GROUPW : g * GROUPW + wg, :], start=True, stop=True,
                )
                nc.vector.tensor_scalar(
                    out=lgT_sb[:, g * GROUPW : g * GROUPW + wg, :],
                    in0=lg_ps[:, 0:wg, :], scalar1=bclsT[:, 0:1], scalar2=None,
                    op0=_ALU.add,
                )

            nc.sync.dma_start(hT_d[:, :, :], hT_sb[:, :, :])
            nc.sync.dma_start(lgT_d[:, :, :], lgT_sb[:, :, :])
    return nc


# ---------------------------------------------------------------------------
# Top-level entry
# ---------------------------------------------------------------------------

_BUILT = {}
last_perf = {}


def _get_kernels():
    if "k" not in _BUILT:
        nc1 = build_l1()
        nc1.compile()
        nc2 = build_l2()
        nc2.compile()
        _BUILT["k"] = (nc1, nc2)
    return _BUILT["k"]


def kernel(**inputs):
    prep = _prepare(inputs)
    node_at_pos = prep["node_at_pos"]
    nc1, nc2 = _get_kernels()
    trace = os.environ.get("GNN_TRACE", "") == "1"

    emb = np.ascontiguousarray(
        np.asarray(inputs["emb"], dtype=np.float32).reshape(NF * V, D)
    )

    # ---- launch 1: build x (natural node order) ----
    in_maps1 = [
        {"emb": emb, "idx16": np.ascontiguousarray(prep["idx16"][c])}
        for c in range(NCORES)
    ]
    res1 = run_bass_kernel_spmd(nc1, in_maps1, core_ids=list(range(NCORES)),
                                trace=trace)
    last_perf["l1"] = res1
    x_tab = np.zeros((NTAB, D), np.float32)
    for c in range(NCORES):
        xo = res1.results[c]["xout"]  # [P, 49, D]
        rows = xo.transpose(1, 0, 2).reshape(POS_PER_CORE, D)
        lo = c * POS_PER_CORE
        hi = min((c + 1) * POS_PER_CORE, N)
        if hi > lo:
            x_tab[lo:hi] = rows[: hi - lo]

    # ---- launch 2: main compute ----
    basis = np.ascontiguousarray(np.asarray(inputs["basis"], dtype=np.float32))
    coeff = np.ascontiguousarray(
        np.asarray(inputs["coeff"], dtype=np.float32).reshape(1, R * B)
    )
    wself = np.ascontiguousarray(np.asarray(inputs["W_self"], dtype=np.float32))
    brel = np.ascontiguousarray(np.asarray(inputs["b_rel"], dtype=np.float32))
    hbias = np.ascontiguousarray(
        np.asarray(inputs["h_bias"], dtype=np.float32).reshape(1, D)
    )
    wcls = np.ascontiguousarray(np.asarray(inputs["W_cls"], dtype=np.float32))
    bcls = np.ascontiguousarray(
        np.asarray(inputs["b_cls"], dtype=np.float32).reshape(1, OUT)
    )

    node_grid = node_at_pos.reshape(WIN_TOTAL, P)
    in_maps2 = []
    for c in range(NCORES):
        nodes = node_grid[c * WIN_PER_CORE : (c + 1) * WIN_PER_CORE]  # [49, p]
        xwin = np.zeros((WIN_PER_CORE, P, D), np.float32)
        mask = nodes >= 0
        xwin[mask] = x_tab[nodes[mask]]
        in_maps2.append(
            {
                "x": x_tab,
                "xwin": np.ascontiguousarray(xwin.transpose(1, 0, 2)),
                "srclo": np.ascontiguousarray(prep["srclo"][c]),
                "srchi": np.ascontiguousarray(prep["srchi"][c]),
                "dst": np.ascontiguousarray(prep["dstloc"][c]),
                "basis": basis,
                "coeff": coeff,
                "wself": wself,
                "brel": brel,
                "hbias": hbias,
                "wcls": wcls,
                "bcls": bcls,
            }
        )
    res2 = run_bass_kernel_spmd(nc2, in_maps2, core_ids=list(range(NCORES)),
                                trace=trace)
    last_perf["l2"] = res2

    h_full = np.zeros((N, D), np.float32)
    lg_full = np.zeros((N, OUT), np.float32)
    for c in range(NCORES):
        hT = res2.results[c]["hT"]  # [P(o), 49, P(s)]
        lgT = res2.results[c]["lgT"]
        hp = hT.transpose(1, 2, 0).reshape(POS_PER_CORE, D)
        lp = lgT.transpose(1, 2, 0).reshape(POS_PER_CORE, OUT)
        nodes = node_grid[c * WIN_PER_CORE : (c + 1) * WIN_PER_CORE].reshape(-1)
        mask = nodes >= 0
        h_full[nodes[mask]] = hp[mask]
        lg_full[nodes[mask]] = lp[mask]
    return lg_full, h_full


# revision 13
# speedup vs baseline: 2.1843x; 2.1843x over previous
"""Trainium2 Bass kernel for nn_Entity_Classify (RGCN-style entity classification).

Algorithm (8 NeuronCores, SPMD):
  - Destination nodes are permuted into 392 windows of 128 slots, balanced so
    every (relation, window, src-half) has <= 128 incoming edges. Each core owns
    49 consecutive windows for ALL 8 relations -> no output collective needed.
  - Launch 1: each core builds its shard of x = sum_f emb[f][codes[f]] via a
    dma_gather from the flattened embedding table; host assembles the full x
    table in natural node order.
  - Launch 2 (per core, per window group): dma_gather x[src] for the group's
    edges (two gathers: src<25088 from the table head, src>=25088 from an
    offset view, keeping int16 indices in range); build one-hot H from dst
    slots on-device (iota + is_equal); segment-sum via PE matmuls
    msg = H^T @ xg and deg = H^T @ 1 (H stays weight-stationary);
    mean = msg * 1/max(deg,1) as a per-partition scale; PE transpose;
    h_pre^T accumulates sum_r Wn_r^T @ mean_r^T + Ws_sum^T @ x^T in PSUM;
    relu(+bias) on ACT; classifier matmul; host un-permutes the outputs.
  All floating-point math (embedding sum, W_neigh = coeff@basis, Ws_sum,
  biases, segment mean, matmuls, relu, classifier) happens on device.
"""

import os
import sys

import numpy as np

try:
    import concourse.bass as bass  # noqa: F401
except ImportError:  # pragma: no cover
    sys.path.insert(0, "/opt/trn_rl_repo")
    import concourse.bass as bass  # noqa: F401

import concourse.mybir as mybir
import concourse.tile as tile
from concourse import bacc
from concourse.bass_utils import run_bass_kernel_spmd
from concourse.masks import make_identity

F32 = mybir.dt.float32
F16 = mybir.dt.float16
I32 = mybir.dt.int32
I16 = mybir.dt.int16

# Problem sizes (hardcoded from the spec).
N = 50000
D = 128
R = 8
E = 80000
B = 4
NF = 4
V = 1000
OUT = 16

NCORES = 8
P = 128
WIN_TOTAL = 392
WIN_PER_CORE = 49
N_PAD = WIN_TOTAL * P  # 50176 positions
POS_PER_CORE = WIN_PER_CORE * P  # 6272
GROUPW = 3  # windows per group in the main loop
NGROUPS = 17  # 16 groups of 3 windows + 1 group of 1 window
LASTW = WIN_PER_CORE - GROUPW * (NGROUPS - 1)  # 1

# x table (natural node order) and the int16-safe lo/hi split.
NTAB = 50048  # 50000 nodes + zero pad rows
SPLIT = 25088  # src < SPLIT -> lo gather, else hi gather
HIOFF = NTAB - 32768  # 17280; hi gather reads x[HIOFF:HIOFF+32768]

_ALU = mybir.AluOpType
_ACT = mybir.ActivationFunctionType


def _group_w(g):
    return GROUPW if g < NGROUPS - 1 else LASTW


def _col_layout():
    """Per-group column bases. Per group: 2*R*wg chunk columns
    (lo chunks first: r*wg+wi, then hi chunks at R*wg + r*wg+wi)."""
    bases = []
    acc = 0
    for g in range(NGROUPS):
        bases.append(acc)
        acc += 2 * R * _group_w(g)
    return bases, acc


COL_BASES, NCOLS = _col_layout()  # NCOLS == 784
# int16 index-column offsets per group per half (16 idxs per column).
SOFF = [R * _group_w(g) * 8 for g in range(NGROUPS)]
SOFF = [0] + list(np.cumsum(SOFF))
S_TOT = SOFF[-1]  # 3136


def _pack16(dst, j_arr, s_off):
    """Pack indices j_arr (order j) into int16 grid at [16k + j%16, s_off + j//16],
    replicated across the 8 Q7 partition groups."""
    jpos = np.arange(len(j_arr))
    for k in range(8):
        dst[16 * k + jpos % 16, s_off + jpos // 16] = j_arr


# ---------------------------------------------------------------------------
# Host-side preparation (integer index bookkeeping only)
# ---------------------------------------------------------------------------


def _assign_positions(deg, cap):
    """Assign nodes to padded positions so each window's per-column count of
    `deg` [N, K] stays <= cap. Returns node_at_pos [N_PAD] (-1 = empty)."""
    K = deg.shape[1]
    deg_pad = np.vstack([deg, np.zeros((N_PAD - N, K), np.int64)])
    rng = np.random.default_rng(12345)
    for _attempt in range(8):
        perm = rng.permutation(N)
        node_at_pos = np.full(N_PAD, -1, np.int64)
        node_at_pos[:N] = perm
        d2 = np.zeros((N_PAD, K), np.int64)
        d2[:N] = deg[perm]
        cnt = d2.reshape(WIN_TOTAL, P, K).sum(1)

        ok = True
        for _it in range(6000):
            over = cnt.max(1) - cap
            w1 = int(np.argmax(over))
            if over[w1] <= 0:
                break
            r1 = int(np.argmax(cnt[w1]))
            nd = node_at_pos[w1 * P : (w1 + 1) * P]
            degs1 = np.where(nd >= 0, deg_pad[np.clip(nd, 0, None), r1], -1)
            p1 = int(np.argmax(degs1))
            n1 = nd[p1]
            dn1 = deg_pad[n1]
            newmax = (cnt + dn1).max(1)
            newmax[w1] = 1 << 30
            w2 = int(np.argmin(newmax))
            nd2 = node_at_pos[w2 * P : (w2 + 1) * P]
            degs2 = np.where(nd2 >= 0, deg_pad[np.clip(nd2, 0, None), r1], 0)
            p2 = int(np.argmin(degs2))
            n2 = nd2[p2]
            dn2 = deg_pad[n2] if n2 >= 0 else np.zeros(K, np.int64)
            node_at_pos[w1 * P + p1] = n2
            node_at_pos[w2 * P + p2] = n1
            cnt[w1] += dn2 - dn1
            cnt[w2] += dn1 - dn2
        else:
            ok = False
        if ok and (cnt <= cap).all():
            return node_at_pos
    raise RuntimeError("window balancing failed")


def _prepare(inputs):
    """All host-side integer bookkeeping."""
    feat_codes = np.asarray(inputs["feat_codes"])
    nid = np.asarray(inputs["nid"])
    edges_src = np.asarray(inputs["edges_src"])
    edges_dst = np.asarray(inputs["edges_dst"])

    # per-node degree split by (relation, src-half)
    deg = np.zeros((N, 2 * R), np.int64)
    for r in range(R):
        half = (np.asarray(edges_src[r]) >= SPLIT).astype(np.int64)
        np.add.at(deg[:, 2 * r], edges_dst[r][half == 0], 1)
        np.add.at(deg[:, 2 * r + 1], edges_dst[r][half == 1], 1)
    node_at_pos = _assign_positions(deg, P)

    pos_of_node = np.full(N, -1, np.int64)
    valid = node_at_pos >= 0
    pos_of_node[node_at_pos[valid]] = np.nonzero(valid)[0]

    # Packed edge arrays per core.
    srclo = np.zeros((NCORES, P, S_TOT), np.int16)
    srchi = np.zeros((NCORES, P, S_TOT), np.int16)
    dstloc = np.full((NCORES, P, NCOLS), -1.0, np.float32)
    for r in range(R):
        src = np.asarray(edges_src[r]).astype(np.int64)
        dpos = pos_of_node[edges_dst[r]]
        w = dpos // P
        slot = (dpos % P).astype(np.float32)
        half = (src >= SPLIT).astype(np.int64)
        order = np.lexsort((half, w))
        ws = w[order]
        bounds = np.searchsorted(ws, np.arange(WIN_TOTAL + 1))
        for wglob in range(WIN_TOTAL):
            lo_, hi_ = bounds[wglob], bounds[wglob + 1]
            if lo_ == hi_:
                continue
            core, wl = divmod(wglob, WIN_PER_CORE)
            g = min(wl // GROUPW, NGROUPS - 1)
            wi = wl - g * GROUPW
            wg = _group_w(g)
            ed = order[lo_:hi_]
            hmask = half[ed]
            for h, (arr, bias) in enumerate(((srclo, 0), (srchi, HIOFF))):
                seg = ed[hmask == h]
                assert len(seg) <= P, f"window overflow {wglob} r={r} h={h}"
                if len(seg) == 0:
                    continue
                cwh = r * wg + wi  # chunk-column within this half
                jpos = cwh * P + np.arange(len(seg))
                vals = (src[seg] - bias).astype(np.int16)
                for k in range(8):
                    arr[core, 16 * k + jpos % 16, SOFF[g] + jpos // 16] = vals
                col = COL_BASES[g] + h * R * wg + cwh
                dstloc[core, : len(seg), col] = slot[seg]

    # Launch-1 gather indices (natural node order, flat emb row f*V + code).
    codes = feat_codes[:, nid].astype(np.int64)  # [NF, N]
    idx16 = np.zeros((NCORES, P, NF * WIN_PER_CORE * 8), np.int16)
    for core in range(NCORES):
        base_node = core * POS_PER_CORE
        nodes = base_node + np.arange(POS_PER_CORE)
        ok = nodes < N
        safe = np.clip(nodes, 0, N - 1)
        for f in range(NF):
            vals = np.where(ok, f * V + codes[f][safe], 0)
            # j = (f*49 + c)*128 + p  with node = base + c*128 + p
            _pack16(idx16[core], vals.astype(np.int16), f * WIN_PER_CORE * 8)
    return dict(
        node_at_pos=node_at_pos,
        srclo=srclo,
        srchi=srchi,
        dstloc=dstloc,
        idx16=idx16,
    )


# ---------------------------------------------------------------------------
# Launch 1: embedding build (x shard per core, natural node order)
# ---------------------------------------------------------------------------


def build_l1():
    nc = bacc.Bacc(None, target_bir_lowering=False,
                   dynamic_dma_scratch_size=16384, num_swdge_queues=2)
    emb_d = nc.dram_tensor("emb", [NF * V, D], F32, kind="ExternalInput")
    idx_d = nc.dram_tensor("idx16", [P, NF * WIN_PER_CORE * 8], I16,
                           kind="ExternalInput")
    out_d = nc.dram_tensor("xout", [P, WIN_PER_CORE, D], F16,
                           kind="ExternalOutput")

    W = WIN_PER_CORE
    with tile.TileContext(nc) as tc:
        with tc.tile_pool(name="sb", bufs=1) as sb:
            idx_t = sb.tile([P, NF * W * 8], I16)
            nc.sync.dma_start(idx_t[:, :], idx_d[:, :])
            g_t = sb.tile([P, NF * W, D], F32)
            for f in range(NF):
                nc.gpsimd.dma_gather(
                    out_ap=g_t[:, f * W : (f + 1) * W, :], in_ap=emb_d[:, :],
                    idxs_ap=idx_t[:, f * W * 8 : (f + 1) * W * 8],
                    num_idxs=W * P, num_idxs_reg=W * P, elem_size=D,
                    single_packet=False, queue_num=f % 2,
                )
            t01 = sb.tile([P, W, D], F32)
            t23 = sb.tile([P, W, D], F32)
            x_t = sb.tile([P, W, D], F16)
            nc.vector.tensor_tensor(
                out=t01[:, :, :], in0=g_t[:, 0:W, :], in1=g_t[:, W : 2 * W, :],
                op=_ALU.add,
            )
            nc.vector.tensor_tensor(
                out=t23[:, :, :], in0=g_t[:, 2 * W : 3 * W, :],
                in1=g_t[:, 3 * W : 4 * W, :], op=_ALU.add,
            )
            nc.vector.tensor_tensor(
                out=x_t[:, :, :], in0=t01[:, :, :], in1=t23[:, :, :], op=_ALU.add
            )
            # reuse the gather buffer region for output staging is not needed;
            # write x_t through the first W blocks of out_d
            nc.sync.dma_start(out_d[:, :, :], x_t[:, :, :])
    return nc


# ---------------------------------------------------------------------------
# Launch 2: main compute
# ---------------------------------------------------------------------------


def build_l2():
    nc = bacc.Bacc(None, target_bir_lowering=False,
                   dynamic_dma_scratch_size=16384, num_swdge_queues=2)
    x_d = nc.dram_tensor("x", [NTAB, D], F16, kind="ExternalInput")
    xwin_d = nc.dram_tensor("xwin", [P, WIN_PER_CORE, D], F16, kind="ExternalInput")
    srclo_d = nc.dram_tensor("srclo", [P, S_TOT], I16, kind="ExternalInput")
    srchi_d = nc.dram_tensor("srchi", [P, S_TOT], I16, kind="ExternalInput")
    dst_d = nc.dram_tensor("dst", [P, NCOLS], F32, kind="ExternalInput")
    basis_d = nc.dram_tensor("basis", [B, D, D], F32, kind="ExternalInput")
    coeff_d = nc.dram_tensor("coeff", [1, R * B], F32, kind="ExternalInput")
    wself_d = nc.dram_tensor("wself", [R, D, D], F32, kind="ExternalInput")
    brel_d = nc.dram_tensor("brel", [R, D], F32, kind="ExternalInput")
    hbias_d = nc.dram_tensor("hbias", [1, D], F32, kind="ExternalInput")
    wcls_d = nc.dram_tensor("wcls", [D, OUT], F32, kind="ExternalInput")
    bcls_d = nc.dram_tensor("bcls", [1, OUT], F32, kind="ExternalInput")
    hT_d = nc.dram_tensor("hT", [P, WIN_PER_CORE, P], F32, kind="ExternalOutput")
    lgT_d = nc.dram_tensor("lgT", [OUT, WIN_PER_CORE, P], F32, kind="ExternalOutput")

    # supergroups: gather granularity (2 groups per gather pair)
    SGS = [(0, 1), (2, 3), (4, 5), (6, 7), (8, 9), (10, 11), (12, 13), (14, 15),
           (16,)]

    with tile.TileContext(nc) as tc:
        with (
            tc.tile_pool(name="const", bufs=1) as cpool,
            tc.tile_pool(name="xg", bufs=2) as xgpool,
            tc.tile_pool(name="hh", bufs=2) as hpool,
            tc.tile_pool(name="small", bufs=3) as spool,
            tc.tile_pool(name="mean", bufs=2) as mpool,
            tc.tile_pool(name="mts", bufs=2) as mtspool,
            tc.tile_pool(name="pmsg", bufs=2, space="PSUM") as pmsg,
            tc.tile_pool(name="pmt", bufs=2, space="PSUM") as pmt,
            tc.tile_pool(name="ph", bufs=2, space="PSUM") as ph,
            tc.tile_pool(name="plg", bufs=2, space="PSUM") as plg,
        ):
            # ---- constants / weights prep ----
            ident = cpool.tile([P, P], F16)
            make_identity(nc, ident[:, :])
            iota_i = cpool.tile([P, P], I32)
            nc.gpsimd.iota(iota_i[:, :], pattern=[[1, P]], base=0, channel_multiplier=0)
            iota_f = cpool.tile([P, P], F32)
            nc.vector.tensor_copy(iota_f[:, :], iota_i[:, :])

            srclo_t = cpool.tile([P, S_TOT], I16)
            srchi_t = cpool.tile([P, S_TOT], I16)
            dst_t = cpool.tile([P, NCOLS], F32)
            nc.sync.dma_start(srclo_t[:, :], srclo_d[:, :])
            nc.sync.dma_start(srchi_t[:, :], srchi_d[:, :])
            nc.sync.dma_start(dst_t[:, :], dst_d[:, :])

            basis_t = cpool.tile([P, B, D], F32)
            nc.sync.dma_start(basis_t[:, :, :], basis_d.rearrange("b i o -> i b o"))
            wself_t = cpool.tile([P, R, D], F32)
            nc.sync.dma_start(wself_t[:, :, :], wself_d.rearrange("r i o -> i r o"))
            coeff_t = cpool.tile([1, R * B], F32)
            nc.sync.dma_start(coeff_t[:, :], coeff_d[:, :])
            brel_t = cpool.tile([R, D], F32)
            nc.sync.dma_start(brel_t[:, :], brel_d[:, :])
            hbias_t = cpool.tile([1, D], F32)
            nc.sync.dma_start(hbias_t[:, :], hbias_d[:, :])
            wcls_t = cpool.tile([D, OUT], F32)
            nc.sync.dma_start(wcls_t[:, :], wcls_d[:, :])
            bcls_t = cpool.tile([1, OUT], F32)
            nc.sync.dma_start(bcls_t[:, :], bcls_d[:, :])

            ones_row = cpool.tile([1, P], F32)
            nc.vector.memset(ones_row[:, :], 1.0)
            ones_col = cpool.tile([P, 1], F16)
            nc.vector.memset(ones_col[:, :], 1.0)
            ones8 = cpool.tile([R, 1], F32)
            nc.vector.memset(ones8[:, :], 1.0)
            one1 = cpool.tile([1, 1], F32)
            nc.vector.memset(one1[:, :], 1.0)

            # broadcast coeff to all partitions via ones matmul
            cb_ps = pmt.tile([P, R * B], F32, tag="mt")
            nc.tensor.matmul(cb_ps[:, :], ones_row[:, :], coeff_t[:, :],
                             start=True, stop=True)
            coeffb = cpool.tile([P, R * B], F32)
            nc.vector.tensor_copy(coeffb[:, :], cb_ps[:, :])

            # W_neigh[r] = sum_b coeff[r,b]*basis[b] -> fp16 [i, r, o]
            wn32 = cpool.tile([P, D], F32)
            tmp_w = cpool.tile([P, D], F32)
            wn_t = cpool.tile([P, R, D], F16)
            for r in range(R):
                nc.vector.tensor_tensor(
                    out=wn32[:, :], in0=basis_t[:, 0, :],
                    in1=coeffb[:, 4 * r : 4 * r + 1].to_broadcast([P, D]),
                    op=_ALU.mult,
                )
                for b in range(1, B):
                    nc.vector.tensor_tensor(
                        out=tmp_w[:, :], in0=basis_t[:, b, :],
                        in1=coeffb[:, 4 * r + b : 4 * r + b + 1].to_broadcast([P, D]),
                        op=_ALU.mult,
                    )
                    nc.vector.tensor_tensor(
                        out=wn32[:, :], in0=wn32[:, :], in1=tmp_w[:, :], op=_ALU.add
                    )
                nc.vector.tensor_copy(wn_t[:, r, :], wn32[:, :])

            # Ws_sum = sum_r W_self[r] -> fp16
            ws32 = cpool.tile([P, D], F32)
            nc.vector.tensor_tensor(
                out=ws32[:, :], in0=wself_t[:, 0, :], in1=wself_t[:, 1, :], op=_ALU.add
            )
            for r in range(2, R):
                nc.vector.tensor_tensor(
                    out=ws32[:, :], in0=ws32[:, :], in1=wself_t[:, r, :], op=_ALU.add
                )
            wssum = cpool.tile([P, D], F16)
            nc.vector.tensor_copy(wssum[:, :], ws32[:, :])

            # b_total^T [P,1] and b_cls^T [OUT,1]
            bsum_ps = plg.tile([1, D], F32, tag="lg")
            nc.tensor.matmul(bsum_ps[:, :], ones8[:, :], brel_t[:, :],
                             start=True, stop=True)
            bb = cpool.tile([1, D], F32)
            nc.vector.tensor_tensor(
                out=bb[:, :], in0=bsum_ps[:, :], in1=hbias_t[:, :], op=_ALU.add
            )
            btot_ps = pmt.tile([P, 1], F32, tag="mt")
            nc.tensor.matmul(btot_ps[:, :], bb[:, :], one1[:, :], start=True, stop=True)
            btotT = cpool.tile([P, 1], F32)
            nc.vector.tensor_copy(btotT[:, :], btot_ps[:, :])
            bcls_ps = plg.tile([OUT, 1], F32, tag="lg")
            nc.tensor.matmul(bcls_ps[:, :], bcls_t[:, :], one1[:, :],
                             start=True, stop=True)
            bclsT = cpool.tile([OUT, 1], F32)
            nc.vector.tensor_copy(bclsT[:, :], bcls_ps[:, :])

            # xT (fp16) for the self path
            xwin_t = hpool.tile([P, WIN_PER_CORE, D], F16, tag="H")
            nc.sync.dma_start(xwin_t[:, :, :], xwin_d[:, :, :])
            xT = cpool.tile([P, WIN_PER_CORE, D], F16)
            for w in range(WIN_PER_CORE):
                tp = pmt.tile([P, GROUPW, P], F16, tag="mt")
                nc.tensor.transpose(tp[:, 0, :], xwin_t[:, w, :], ident[:, :])
                if w % 2 == 0:
                    nc.vector.tensor_copy(xT[:, w, :], tp[:, 0, :])
                else:
                    nc.scalar.activation(xT[:, w, :], tp[:, 0, :], _ACT.Copy)

            hT_sb = cpool.tile([P, WIN_PER_CORE, P], F32)
            lgT_sb = cpool.tile([OUT, WIN_PER_CORE, P], F32)

            # ---- main loop over supergroups / groups ----
            for sg in SGS:
                halfc = sum(R * _group_w(g) for g in sg)  # chunk cols per half
                nidx = halfc * P
                soff = SOFF[sg[0]]
                xg = xgpool.tile([P, 2 * 2 * R * GROUPW, P], F16, tag="xg")
                nc.gpsimd.dma_gather(
                    out_ap=xg[:, 0:halfc, :], in_ap=x_d[0:32768, :],
                    idxs_ap=srclo_t[:, soff : soff + halfc * 8],
                    num_idxs=nidx, num_idxs_reg=nidx, elem_size=D,
                    single_packet=False, queue_num=0,
                )
                nc.gpsimd.dma_gather(
                    out_ap=xg[:, halfc : 2 * halfc, :],
                    in_ap=x_d[HIOFF : HIOFF + 32768, :],
                    idxs_ap=srchi_t[:, soff : soff + halfc * 8],
                    num_idxs=nidx, num_idxs_reg=nidx, elem_size=D,
                    single_packet=False, queue_num=1,
                )

                for gi, g in enumerate(sg):
                    wg = _group_w(g)
                    nchunk = R * wg
                    base = COL_BASES[g]
                    goff = sum(R * _group_w(g2) for g2 in sg[:gi])

                    H = hpool.tile([P, 2 * R * GROUPW, P], F16, tag="H")
                    for h in range(2):
                        # batched one-hot build: H[e, lh, s] = (iota[s]==dst)
                        iap = iota_f[:, :]
                        iota_b = bass.AP(
                            iap.tensor, iap.offset,
                            [iap.ap[0], [0, nchunk], iap.ap[1]],
                        )
                        nc.vector.tensor_tensor(
                            out=H[:, h * nchunk : (h + 1) * nchunk, :],
                            in0=iota_b,
                            in1=dst_t[
                                :, base + h * nchunk : base + (h + 1) * nchunk
                            ].to_broadcast([P, nchunk, P]),
                            op=_ALU.is_equal,
                        )

                    h_ps = ph.tile([P, GROUPW, P], F32, tag="h")
                    nc.tensor.matmul(
                        h_ps[:, 0:wg, :], wssum[:, :],
                        xT[:, g * GROUPW : g * GROUPW + wg, :], start=True, stop=False,
                    )

                    for r in range(R):
                        msg = pmsg.tile([P, GROUPW, 129], F32, tag="msg")
                        for wi in range(wg):
                            ccol = r * wg + wi
                            xlo = goff + ccol
                            xhi = halfc + goff + ccol
                            hlo = ccol
                            hhi = nchunk + ccol
                            nc.tensor.matmul(
                                msg[:, wi, 0:128], H[:, hlo, :], xg[:, xlo, :],
                                start=(wi == 0), stop=False, skip_group_check=True,
                            )
                            nc.tensor.matmul(
                                msg[:, wi, 128:129], H[:, hlo, :], ones_col[:, :],
                                start=False, stop=False, skip_group_check=True,
                            )
                            nc.tensor.matmul(
                                msg[:, wi, 0:128], H[:, hhi, :], xg[:, xhi, :],
                                start=False, stop=False, skip_group_check=True,
                            )
                            nc.tensor.matmul(
                                msg[:, wi, 128:129], H[:, hhi, :], ones_col[:, :],
                                start=False, stop=(wi == wg - 1),
                                skip_group_check=True,
                            )
                        degm = spool.tile([P, GROUPW], F32, tag="degm")
                        nc.vector.tensor_scalar(
                            out=degm[:, 0:wg], in0=msg[:, 0:wg, 128], scalar1=1.0,
                            scalar2=None, op0=_ALU.max,
                        )
                        recip = spool.tile([P, GROUPW], F32, tag="recip")
                        nc.vector.reciprocal(recip[:, 0:wg], degm[:, 0:wg])
                        mean = mpool.tile([P, GROUPW, P], F16, tag="mean")
                        nc.vector.tensor_tensor(
                            out=mean[:, 0:wg, :], in0=msg[:, 0:wg, 0:128],
                            in1=recip[:, 0:wg].to_broadcast([P, wg, P]), op=_ALU.mult,
                        )
                        mt_ps = pmt.tile([P, GROUPW, P], F16, tag="mt")
                        for wi in range(wg):
                            nc.tensor.transpose(mt_ps[:, wi, :], mean[:, wi, :],
                                                ident[:, :])
                        meanT = mtspool.tile([P, GROUPW, P], F16, tag="mts")
                        if r % 2 == 0:
                            nc.vector.tensor_copy(meanT[:, 0:wg, :], mt_ps[:, 0:wg, :])
                        else:
                            nc.scalar.activation(meanT[:, 0:wg, :], mt_ps[:, 0:wg, :],
                                                 _ACT.Copy)
                        nc.tensor.matmul(
                            h_ps[:, 0:wg, :], wn_t[:, r, :], meanT[:, 0:wg, :],
                            start=False, stop=(r == R - 1),
                        )

                    nc.scalar.activation(
                        hT_sb[:, g * GROUPW : g * GROUPW + wg, :], h_ps[:, 0:wg, :],
                        _ACT.Relu, bias=btotT[:, 0:1], scale=1.0,
                    )
                    lg_ps = plg.tile([OUT, GROUPW, P], F32, tag="lg")
                    nc.tensor.matmul(
                        lg_ps[:, 0:wg, :], wcls_t[:, :],
                        hT_sb[:, g * GROUPW : g * GROUPW + wg, :],
                        start=True, stop=True,
                    )
                    bap = bclsT[:, 0:1]
                    bcls_b = bass.AP(
                        bap.tensor, bap.offset, [bap.ap[0], [0, wg], [0, P]]
                    )
                    nc.vector.tensor_tensor(
                        out=lgT_sb[:, g * GROUPW : g * GROUPW + wg, :],
                        in0=lg_ps[:, 0:wg, :],
                        in1=bcls_b,
                        op=_ALU.add,
                    )

            nc.sync.dma_start(hT_d[:, :, :], hT_sb[:, :, :])
            nc.sync.dma_start(lgT_d[:, :, :], lgT_sb[:, :, :])
    return nc


# ---------------------------------------------------------------------------
# Top-level entry
# ---------------------------------------------------------------------------

_BUILT = {}
last_perf = {}


def _get_kernels():
    if "k" not in _BUILT:
        nc1 = build_l1()
        nc1.compile()
        nc2 = build_l2()
        nc2.compile()
        _BUILT["k"] = (nc1, nc2)
    return _BUILT["k"]


def kernel(**inputs):
    prep = _prepare(inputs)
    node_at_pos = prep["node_at_pos"]
    nc1, nc2 = _get_kernels()
    trace = os.environ.get("GNN_TRACE", "") == "1"

    emb = np.ascontiguousarray(
        np.asarray(inputs["emb"], dtype=np.float32).reshape(NF * V, D)
    )

    # ---- launch 1: build x (natural node order) ----
    in_maps1 = [
        {"emb": emb, "idx16": np.ascontiguousarray(prep["idx16"][c])}
        for c in range(NCORES)
    ]
    res1 = run_bass_kernel_spmd(nc1, in_maps1, core_ids=list(range(NCORES)),
                                trace=trace)
    last_perf["l1"] = res1
    x_tab = np.zeros((NTAB, D), np.float16)
    for c in range(NCORES):
        xo = res1.results[c]["xout"]  # [P, 49, D]
        rows = xo.transpose(1, 0, 2).reshape(POS_PER_CORE, D)
        lo = c * POS_PER_CORE
        hi = min((c + 1) * POS_PER_CORE, N)
        if hi > lo:
            x_tab[lo:hi] = rows[: hi - lo]

    # ---- launch 2: main compute ----
    basis = np.ascontiguousarray(np.asarray(inputs["basis"], dtype=np.float32))
    coeff = np.ascontiguousarray(
        np.asarray(inputs["coeff"], dtype=np.float32).reshape(1, R * B)
    )
    wself = np.ascontiguousarray(np.asarray(inputs["W_self"], dtype=np.float32))
    brel = np.ascontiguousarray(np.asarray(inputs["b_rel"], dtype=np.float32))
    hbias = np.ascontiguousarray(
        np.asarray(inputs["h_bias"], dtype=np.float32).reshape(1, D)
    )
    wcls = np.ascontiguousarray(np.asarray(inputs["W_cls"], dtype=np.float32))
    bcls = np.ascontiguousarray(
        np.asarray(inputs["b_cls"], dtype=np.float32).reshape(1, OUT)
    )

    node_grid = node_at_pos.reshape(WIN_TOTAL, P)
    in_maps2 = []
    for c in range(NCORES):
        nodes = node_grid[c * WIN_PER_CORE : (c + 1) * WIN_PER_CORE]  # [49, p]
        xwin = np.zeros((WIN_PER_CORE, P, D), np.float16)
        mask = nodes >= 0
        xwin[mask] = x_tab[nodes[mask]]
        in_maps2.append(
            {
                "x": x_tab,
                "xwin": np.ascontiguousarray(xwin.transpose(1, 0, 2)),
                "srclo": np.ascontiguousarray(prep["srclo"][c]),
                "srchi": np.ascontiguousarray(prep["srchi"][c]),
                "dst": np.ascontiguousarray(prep["dstloc"][c]),
                "basis": basis,
                "coeff": coeff,
                "wself": wself,
                "brel": brel,
                "hbias": hbias,
                "wcls": wcls,
                "bcls": bcls,
            }
        )
    res2 = run_bass_kernel_spmd(nc2, in_maps2, core_ids=list(range(NCORES)),
                                trace=trace)
    last_perf["l2"] = res2

    h_full = np.zeros((N, D), np.float32)
    lg_full = np.zeros((N, OUT), np.float32)
    for c in range(NCORES):
        hT = res2.results[c]["hT"]  # [P(o), 49, P(s)]
        lgT = res2.results[c]["lgT"]
        hp = hT.transpose(1, 2, 0).reshape(POS_PER_CORE, D)
        lp = lgT.transpose(1, 2, 0).reshape(POS_PER_CORE, OUT)
        nodes = node_grid[c * WIN_PER_CORE : (c + 1) * WIN_PER_CORE].reshape(-1)
        mask = nodes >= 0
        h_full[nodes[mask]] = hp[mask]
        lg_full[nodes[mask]] = lp[mask]
    return lg_full, h_full
